# revision 10
# baseline (speedup 1.0000x reference)
"""Trainium2 Bass kernel for nn_Attention_msa_visual (dense_transformer).

Contract: kernel(**inputs) takes FULL inputs, returns FULL outputs
(x_cls_out (1,N,2C), sim_round2 (N,N)) — matching reference.reference.

Dead-code elimination: only v_cls = x_cls @ W_v^T feeds the returned
outputs (q/k of both streams, v_reg, cls_score, fg_score are dead).

Sharding: one head per NeuronCore (8 heads / 8 cores). Each core:
  v_h = x @ Wv_h^T                      (PE, fp32)
  vn_h = v_h / ||v_h||                  (ACT Square+Sqrt, DVE recip)
  raw_h = vn_h @ vn_h^T (16 row chunks) (PE)
  E = exp(25*raw), rowsum fused         (ACT accum_out)
  attn_h = E / rowsum  -> bf16          (DVE per-partition scalar)
  x_h = (E @ v_h) / rowsum              (PE, symmetry: E^T slice == E slice)
Host gathers raw_h/attn_h/x_h/v_h and does the cheap head-sum +
mask + renormalize during unshard.
"""

import sys

for _p in ("/opt/trn_rl_repo",):
    if _p not in sys.path:
        sys.path.insert(0, _p)

import numpy as np

import concourse.bass as bass
import concourse.bacc as bacc
import concourse.mybir as mybir
import concourse.tile as tile
from concourse.bass_utils import run_bass_kernel_spmd

N, C, H, HD = 2048, 1024, 8, 128
P = 128
NCH = N // P     # 16 row chunks
CCH = C // P     # 8 contraction chunks
RAW_SCALE = 25.0
SIM_THRESH = 0.75

LAST_RESULT = None
_NC_CACHE = None


def _build():
    f32 = mybir.dt.float32
    bf16 = mybir.dt.bfloat16
    nc = bacc.Bacc("TRN2")

    # fused input: per c-chunk columns [0:N]=x^T, [N:N+HD]=Wv^T, [N+HD:]=I
    XW = N + HD + P
    xw = nc.dram_tensor("xw", (C, XW), f32, kind="ExternalInput")
    raw_o = nc.dram_tensor("raw", (N, N), bf16, kind="ExternalOutput")
    attn_o = nc.dram_tensor("attn", (N, N), bf16, kind="ExternalOutput")
    x_o = nc.dram_tensor("xh", (N, HD), f32, kind="ExternalOutput")
    v_o = nc.dram_tensor("vh", (N, HD), f32, kind="ExternalOutput")

    AF = mybir.ActivationFunctionType

    with tile.TileContext(nc) as tc:
        with (
            tc.tile_pool(name="sb", bufs=1) as sb,
            tc.tile_pool(name="esc", bufs=2) as esc,
            tc.tile_pool(name="asc", bufs=2) as asc,
            tc.tile_pool(name="xsc", bufs=2) as xsc,
            tc.tile_pool(name="ps", bufs=1, space="PSUM") as psp,
        ):
            xt_sb = sb.tile([P, CCH, XW], f32)    # fused x^T/Wv^T/I chunks
            vnat = sb.tile([P, NCH, HD], f32)     # v natural       1 MB
            vnT = sb.tile([P, N], f32)            # vn^T            1 MB
            normsq = sb.tile([P, NCH], f32)
            rnrm = sb.tile([P, NCH], f32)         # 1/||v||
            rowsum = sb.tile([P, NCH], f32)
            rinv = sb.tile([P, NCH], f32)         # 1/rowsum
            zero_b = sb.tile([P, 1], f32)         # DVE-owned zero bias
            xacc = sb.tile([P, N], f32)           # x' accumulator (SBUF)
            xw_r = xw.rearrange("(co p) n -> p co n", p=P)
            for c in range(CCH):
                nc.sync.dma_start(xt_sb[:, c, :], xw_r[:, c, :])
            ident = xt_sb[:, 0, N + HD:N + HD + P]

            psum_a = psp.tile([P, N], f32, tag="psum_a")  # 4 banks
            psum_b = psp.tile([P, N], f32, tag="psum_b")  # 4 banks

            # ---- projection: v chunk j -> psum_b[:, j*HD:(j+1)*HD] ----
            for j in range(NCH):
                for c in range(CCH):
                    nc.tensor.matmul(
                        psum_b[:, j * HD:(j + 1) * HD],
                        lhsT=xt_sb[:, c, j * P:(j + 1) * P],
                        rhs=xt_sb[:, c, N:N + HD],
                        start=(c == 0),
                        stop=(c == CCH - 1),
                    )

            # ---- normalize + transpose ----
            nc.vector.memset(zero_b, 0.0)
            sqs = sb.tile([P, HD], f32, tag="sqscratch")
            for j in range(NCH):
                nc.vector.tensor_copy(vnat[:, j, :], psum_b[:, j * HD:(j + 1) * HD])
                nc.vector.tensor_mul(sqs, vnat[:, j, :], vnat[:, j, :])
                nc.vector.reduce_sum(normsq[:, j:j + 1], sqs,
                                     axis=mybir.AxisListType.X)
            nc.scalar.activation(rnrm, normsq, AF.Sqrt, bias=zero_b)
            nc.vector.reciprocal(rnrm, rnrm)
            nc.gpsimd.dma_start(v_o.rearrange("(no p) d -> p no d", p=P), vnat)

            for j in range(NCH):
                vns = xsc.tile([P, HD], f32, tag="vnscratch")
                nc.vector.tensor_scalar_mul(vns, vnat[:, j, :], rnrm[:, j:j + 1])
                nc.tensor.transpose(psum_a[:, j * P:(j + 1) * P], vns, ident)
            nc.vector.tensor_copy(vnT, psum_a)

            # warmup Exp: absorbs the exp table-set load wait (DVE dep only)
            nc.scalar.activation(sqs[:, 0:1], zero_b, AF.Exp, bias=zero_b)

            # ---- main loop over row chunks ----
            for m in range(NCH):
                for t in range(4):
                    nc.tensor.matmul(
                        psum_a[:, t * 512:(t + 1) * 512],
                        lhsT=vnT[:, m * P:(m + 1) * P],
                        rhs=vnT[:, t * 512:(t + 1) * 512],
                        start=True,
                        stop=True,
                    )
                e_m = esc.tile([P, N], f32, tag="e")
                nc.scalar.activation(
                    e_m, psum_a, AF.Exp, scale=RAW_SCALE, bias=zero_b,
                    accum_out=rowsum[:, m:m + 1],
                )
                r_m = asc.tile([P, N], bf16, tag="rawcast")
                nc.scalar.activation(r_m, psum_a, AF.Copy)
                nc.gpsimd.dma_start(raw_o[m * P:(m + 1) * P, :], r_m)
                nc.vector.reciprocal(rinv[:, m:m + 1], rowsum[:, m:m + 1])
                a_m = asc.tile([P, N], bf16, tag="a")
                nc.vector.tensor_scalar_mul(a_m, e_m, rinv[:, m:m + 1])
                nc.gpsimd.dma_start(attn_o[m * P:(m + 1) * P, :], a_m)
                # x partials: psum_b[:, i*HD:] = E_m[:, i-slice]^T @ v_m
                for i in range(NCH):
                    nc.tensor.matmul(
                        psum_b[:, i * HD:(i + 1) * HD],
                        lhsT=e_m[:, i * P:(i + 1) * P],
                        rhs=vnat[:, m, :],
                        start=True,
                        stop=True,
                    )
                if m == 0:
                    nc.vector.tensor_copy(xacc, psum_b)
                else:
                    nc.vector.tensor_add(xacc, xacc, psum_b)

            # ---- finalize x = accum / rowsum ----
            for i in range(NCH):
                xs = xsc.tile([P, HD], f32, tag="xfin")
                nc.vector.tensor_scalar_mul(
                    xs, xacc[:, i * HD:(i + 1) * HD], rinv[:, i:i + 1]
                )
                nc.gpsimd.dma_start(x_o[i * P:(i + 1) * P, :], xs)

    nc.compile()
    return nc


def kernel(**inputs):
    global LAST_RESULT, _NC_CACHE
    x = np.ascontiguousarray(np.asarray(inputs["x_cls"], dtype=np.float32)[0])
    W = np.asarray(inputs["W_qkv_cls"], dtype=np.float32)

    xT = x.T                                            # (C, N)
    Wv = W[2 * C:3 * C, :]                              # (C, C) rows = d
    XW = N + HD + P
    in_maps = []
    for h in range(H):
        fused = np.zeros((C, XW), np.float32)
        fused[:, :N] = xT
        fused[:, N:N + HD] = Wv[h * HD:(h + 1) * HD, :].T
        fused[:P, N + HD:] = np.eye(P, dtype=np.float32)
        in_maps.append({"xw": fused})

    if _NC_CACHE is None:
        _NC_CACHE = _build()
    res = run_bass_kernel_spmd(_NC_CACHE, in_maps, core_ids=list(range(H)))
    LAST_RESULT = res
    results = res.results

    xh = np.concatenate([np.asarray(r["xh"]) for r in results], axis=-1)
    vh = np.concatenate([np.asarray(r["vh"]) for r in results], axis=-1)
    x_cls_out = np.concatenate([xh, vh], axis=-1)[None].astype(np.float32)

    raw_sum = np.zeros((N, N), np.float64)
    attn_sum = np.zeros((N, N), np.float64)
    for r in results:
        raw_sum += np.asarray(r["raw"], dtype=np.float64)
        attn_sum += np.asarray(r["attn"]).astype(np.float64)
    acr = (raw_sum / H).astype(np.float32)
    sim_mask = (acr > SIM_THRESH).astype(np.float32)
    sim_attn = (attn_sum / H).astype(np.float32)
    z = sim_attn - sim_attn.max(axis=-1, keepdims=True)
    e = np.exp(z)
    sr2 = e / e.sum(axis=-1, keepdims=True)
    m = sim_mask * sr2
    sim_round2 = (m / m.sum(axis=-1, keepdims=True)).astype(np.float32)

    return (x_cls_out, sim_round2)
